# revision 2
# baseline (speedup 1.0000x reference)
"""Multi-head attention (B=4, S=2048, D=1024, H=16) on 8 trn2 NeuronCores — v2.

Sharding: batch x head-group (Megatron tensor-parallel). Core c = (b=c//2,
hg=c%2) handles batch b, heads hg*8..hg*8+8 (d-slice 512). No redundant
compute: Wq/Wk/Wv column-sliced, W0 row-sliced; host sums the two partial
outputs per batch (+ b0). bk is dropped entirely (softmax shift-invariant);
bv is folded into the host-side b0 add via b0 + W0_slice @ bv_slice.

Per-core pipeline (all bf16 matmuls, N=512 streams):
  proj:   Q^B/K^B [128, 4dc, 2048] (+bq on Q), V^A [128, 16tc, 8h, 64]
  attn:   per (pair dc, sB, tc): scores 4 MMs (2 row-tiled slots, K=64)
          into st [128, 2048] = [h0s0|h0s1|h1s0|h1s1]; ONE exp act
          [128, 2048] -> E bf16; AV 2 col4 slots (M=32 x 4 col groups)
          accumulating pv[sb] [128, 512] over tc; denominators 1 den4 slot
          (M=1 ones x 4 col groups) into den [128, 512] partitions 0/32/64/96.
  norm:   recip(den) DVE, partition_broadcast Pool, mul -> o_all bf16.
  out:    C = O^B.T @ W0T_slice partial [2048, 1024] f32 -> host sum.
"""

import numpy as np
import ml_dtypes

import concourse.bass as bass  # noqa: F401
import concourse.tile as tile
import concourse.mybir as mybir
from concourse import bacc
from concourse.bass_utils import run_bass_kernel_spmd

BF16 = mybir.dt.bfloat16
F32 = mybir.dt.float32
NP_BF16 = ml_dtypes.bfloat16

D = 1024        # d_model
DS = 512        # per-core d slice (8 heads)
S = 2048        # query rows per core (full seq)
T = 2048        # key rows
H = 8           # heads per core
KC = D // 128   # 8 contraction chunks
DC = DS // 128  # 4 head-pair chunks
TC = T // 128   # 16 t-chunks
TB = T // 512   # 4 t-blocks (proj streaming)
SB = S // 512   # 4 s-blocks of 512
NSB = 2         # s-super-blocks of 1024

DEBUG_RAW = False  # dump raw pv/den of (pair 0, sB 0) instead of o_all
DEBUG_QKV_DIRECT = False  # load Q/K/V from DRAM instead of projecting
DEBUG_NPAIRS = None  # restrict attention to first N pairs


def build(loop_n: int = 1, phases=("p", "b", "c"), interleave: bool = True):
    nc = bacc.Bacc("TRN2", target_bir_lowering=False, debug=False)

    xq = nc.dram_tensor("xq", [D, S], BF16, kind="ExternalInput")
    xk = nc.dram_tensor("xk", [D, T], BF16, kind="ExternalInput")
    xv = nc.dram_tensor("xv", [D, T], BF16, kind="ExternalInput")
    wq = nc.dram_tensor("wq", [D, DS], BF16, kind="ExternalInput")
    wk = nc.dram_tensor("wk", [D, DS], BF16, kind="ExternalInput")
    wv = nc.dram_tensor("wv", [D, DS], BF16, kind="ExternalInput")
    w0 = nc.dram_tensor("w0", [DS, D], BF16, kind="ExternalInput")
    bq = nc.dram_tensor("bq", [DS], F32, kind="ExternalInput")
    qkv_direct = None
    if DEBUG_QKV_DIRECT:
        qkv_direct = (
            nc.dram_tensor("qa_in", [128, DC, S], BF16, kind="ExternalInput"),
            nc.dram_tensor("ka_in", [128, DC, T], BF16, kind="ExternalInput"),
            nc.dram_tensor("va_in", [128, TC, H, 64], BF16, kind="ExternalInput"),
        )
    out = nc.dram_tensor("out", [S, D], F32, kind="ExternalOutput")

    with tile.TileContext(nc) as tc_:
        def body():
            _body(nc, tc_, xq, xk, xv, wq, wk, wv, w0, bq, out,
                  phases=phases, interleave=interleave,
                  qkv_direct=qkv_direct)

        if loop_n == 1:
            body()
        else:
            hint = (
                mybir.EngineType.PE,
                mybir.EngineType.Activation,
                mybir.EngineType.DVE,
                mybir.EngineType.SP,
            )
            with tc_.For_i(0, loop_n, 1, hint_engines=hint):
                body()

    nc.compile()
    return nc


def _body(nc, tc_, xq, xk, xv, wq, wk, wv, w0, bq, out, phases, interleave,
          qkv_direct=None):
    from contextlib import ExitStack

    with ExitStack() as ctx:
        persist = ctx.enter_context(tc_.tile_pool(name="persist", bufs=1))
        wpool = ctx.enter_context(tc_.tile_pool(name="wpool", bufs=1))

        q_all = persist.tile([128, DC, S], BF16, tag="q_all")
        k_all = persist.tile([128, DC, T], BF16, tag="k_all")
        v_all = persist.tile([128, TC, H, 64], BF16, tag="v_all")
        o_all = persist.tile([128, DC, S], BF16, tag="o_all")

        wq_t = wpool.tile([128, KC, DS], BF16, tag="wq")
        nc.sync.dma_start(wq_t[:], wq.ap().rearrange("(c p) d -> p c d", p=128))
        wk_t = wpool.tile([128, KC, DS], BF16, tag="wk")
        nc.sync.dma_start(wk_t[:], wk.ap().rearrange("(c p) d -> p c d", p=128))
        wv_t = wpool.tile([128, KC, DS], BF16, tag="wv")
        nc.sync.dma_start(wv_t[:], wv.ap().rearrange("(c p) d -> p c d", p=128))
        w0_t = wpool.tile([128, DC, D], BF16, tag="w0")
        nc.sync.dma_start(w0_t[:], w0.ap().rearrange("(c p) d -> p c d", p=128))
        bq_t = wpool.tile([128, DC], F32, tag="bq")
        nc.sync.dma_start(bq_t[:], bq.ap().rearrange("(c p) -> p c", p=128))
        ones = wpool.tile([128, 1], BF16, tag="ones")
        nc.vector.memset(ones[:], 1.0)

        xq_r = xq.ap().rearrange("(c p) (sb s) -> sb p c s", p=128, s=512)
        xk_r = xk.ap().rearrange("(c p) (tb t) -> tb p c t", p=128, t=512)
        xv_r = xv.ap().rearrange("(c p) (tb t) -> tb p c t", p=128, t=512)

        # ------------- projection slot emitters (each = one PE slot) -------
        xpool = ctx.enter_context(tc_.tile_pool(name="xpool", bufs=1))
        psA = ctx.enter_context(tc_.tile_pool(name="psA", bufs=1, space="PSUM"))

        xq_blks = [None] * SB
        xk_blks = [None] * TB
        xv_blks = [None] * TB

        def load_xq(sb):
            if xq_blks[sb] is None:
                b_ = xpool.tile([128, KC, 512], BF16, tag="xq", bufs=2,
                                name=f"xq{sb}")
                nc.sync.dma_start(b_[:], xq_r[sb])
                xq_blks[sb] = b_
            return xq_blks[sb]

        def load_xk(tb):
            if xk_blks[tb] is None:
                b_ = xpool.tile([128, KC, 512], BF16, tag="xk", bufs=2,
                                name=f"xk{tb}")
                nc.sync.dma_start(b_[:], xk_r[tb])
                xk_blks[tb] = b_
            return xk_blks[tb]

        def load_xv(tb):
            if xv_blks[tb] is None:
                b_ = xpool.tile([128, KC, 512], BF16, tag="xv", bufs=2,
                                name=f"xv{tb}")
                nc.sync.dma_start(b_[:], xv_r[tb])
                xv_blks[tb] = b_
            return xv_blks[tb]

        def emit_vproj(tc):
            """V^A for t-chunk tc: psum [t128, d512], 8 kc MMs + Pool evac."""
            tb, ti = divmod(tc, 4)
            xv_b = load_xv(tb)
            ps = psA.tile([128, 512], F32, tag="psA", name=f"psV{tc}")
            for kc in range(KC):
                nc.tensor.matmul(
                    ps[:],
                    xv_b[:, kc, ti * 128:(ti + 1) * 128],
                    wv_t[:, kc, :],
                    start=(kc == 0), stop=(kc == KC - 1),
                )
            nc.vector.tensor_copy(
                v_all[:, tc, :, :], ps[:].rearrange("p (h d) -> p h d", d=64))

        def emit_kproj(dc, tb):
            xk_b = load_xk(tb)
            ps = psA.tile([128, 512], F32, tag="psA", name=f"psK{dc}_{tb}")
            for kc in range(KC):
                nc.tensor.matmul(
                    ps[:],
                    wk_t[:, kc, dc * 128:(dc + 1) * 128],
                    xk_b[:, kc, :],
                    start=(kc == 0), stop=(kc == KC - 1),
                )
            nc.vector.tensor_copy(k_all[:, dc, tb * 512:(tb + 1) * 512], ps[:])

        def emit_qproj(dc, sb):
            xq_b = load_xq(sb)
            ps = psA.tile([128, 512], F32, tag="psA", name=f"psQ{dc}_{sb}")
            for kc in range(KC):
                nc.tensor.matmul(
                    ps[:],
                    wq_t[:, kc, dc * 128:(dc + 1) * 128],
                    xq_b[:, kc, :],
                    start=(kc == 0), stop=(kc == KC - 1),
                )
            nc.vector.tensor_scalar_add(
                q_all[:, dc, sb * 512:(sb + 1) * 512], ps[:],
                bq_t[:, dc:dc + 1],
            )

        if "b" not in phases:
            # proj-only ablation
            for tc in range(TC):
                emit_vproj(tc)
            for tb in range(TB):
                for dc in range(DC):
                    emit_kproj(dc, tb)
                xk_blks[tb] = None
            for sb in range(SB):
                for dc in range(DC):
                    emit_qproj(dc, sb)
                xq_blks[sb] = None
            with tc_.tile_pool(name="dump", bufs=1) as dump:
                dt_ = dump.tile([128, 512], F32, tag="dump")
                nc.vector.tensor_copy(dt_[:, 0:256], q_all[:, 0, 0:256])
                nc.vector.tensor_copy(dt_[:, 256:512], k_all[:, 0, 0:256])
                nc.sync.dma_start(out.ap()[0:128, 0:512], dt_[:])
            return

        # Deferred projection work, emitted one MICRO-STEP (single matmul or
        # evacuation) at a time inside the attention loop so PE fill stays
        # matched to the Act-gated steady state (~2 micro-steps per tc).
        def _kq_microsteps(dc):
            for tb in range(TB):
                xk_b = xpool.tile([128, KC, 512], BF16, tag="xk", bufs=2,
                                  name=f"xk{dc}_{tb}")
                nc.sync.dma_start(xk_b[:], xk_r[tb])
                ps = psA.tile([128, 512], F32, tag="psA", name=f"psK{dc}_{tb}")
                for kc in range(KC):
                    yield lambda dc=dc, tb=tb, kc=kc, ps=ps, xk_b=xk_b: (
                        nc.tensor.matmul(
                            ps[:],
                            wk_t[:, kc, dc * 128:(dc + 1) * 128],
                            xk_b[:, kc, :],
                            start=(kc == 0), stop=(kc == KC - 1),
                        ))
                yield lambda dc=dc, tb=tb, ps=ps: nc.vector.tensor_copy(
                    k_all[:, dc, tb * 512:(tb + 1) * 512], ps[:])
            for sb in range(SB):
                xq_b = xpool.tile([128, KC, 512], BF16, tag="xq", bufs=2,
                                  name=f"xq{dc}_{sb}")
                nc.sync.dma_start(xq_b[:], xq_r[sb])
                ps = psA.tile([128, 512], F32, tag="psA", name=f"psQ{dc}_{sb}")
                for kc in range(KC):
                    yield lambda dc=dc, sb=sb, kc=kc, ps=ps, xq_b=xq_b: (
                        nc.tensor.matmul(
                            ps[:],
                            wq_t[:, kc, dc * 128:(dc + 1) * 128],
                            xq_b[:, kc, :],
                            start=(kc == 0), stop=(kc == KC - 1),
                        ))
                yield lambda dc=dc, sb=sb, ps=ps: nc.vector.tensor_scalar_add(
                    q_all[:, dc, sb * 512:(sb + 1) * 512], ps[:],
                    bq_t[:, dc:dc + 1])

        if qkv_direct is not None:
            nc.sync.dma_start(q_all[:], qkv_direct[0].ap())
            nc.sync.dma_start(k_all[:], qkv_direct[1].ap())
            nc.sync.dma_start(v_all[:], qkv_direct[2].ap())
            proj_steps = []
        elif False:
            pass
        else:
          for tc in range(TC):
            emit_vproj(tc)
          if interleave:
            for tb in range(TB):
                emit_kproj(0, tb)
                xk_blks[tb] = None
            for sb in range(SB):
                emit_qproj(0, sb)
                xq_blks[sb] = None
            proj_steps = []
            for dc in range(1, DC):
                proj_steps.extend(_kq_microsteps(dc))
            proj_steps.reverse()  # pop() from the end = original order
          else:
            for tb in range(TB):
                for dc in range(DC):
                    emit_kproj(dc, tb)
                xk_blks[tb] = None
            for sb in range(SB):
                for dc in range(DC):
                    emit_qproj(dc, sb)
                xq_blks[sb] = None
            proj_steps = []

        def pop_proj(n):
            for _ in range(n):
                if not proj_steps:
                    return
                proj_steps.pop()()

        # --------------------------- attention -----------------------------
        with (
            tc_.tile_pool(name="psS", bufs=2, space="PSUM") as psS,
            tc_.tile_pool(name="psPV", bufs=2, space="PSUM") as psPV,
            tc_.tile_pool(name="psDen", bufs=1, space="PSUM") as psDen,
            tc_.tile_pool(name="expp", bufs=10) as expp,
            tc_.tile_pool(name="attn", bufs=2) as attn,
        ):
            norm_steps = []
            for pair in range(DC if DEBUG_NPAIRS is None else DEBUG_NPAIRS):
                # proj slots this pair must finish before the NEXT pair needs
                # them: spread 2 per tc across the pair's 32 tc iterations
                for sB in range(NSB):
                    pvs = [
                        psPV.tile([128, 512], F32, tag="pv",
                                  name=f"pv{pair}_{sB}_{sb}")
                        for sb in range(2)
                    ]
                    den = psDen.tile([128, 512], F32, tag="den",
                                     name=f"den{pair}_{sB}")
                    def emit_av_den(p_es, p_tc, sbq):
                        # col4 AV slot + den2 for one half-tc E tile; the
                        # matmuls wait on that act only, so place them right
                        # before the scores that reuse its st bank.
                        for hh in range(2):
                            h = 2 * pair + hh
                            for half in range(2):
                                g = hh * 2 + half
                                nc.tensor.matmul(
                                    pvs[sbq][32 * g:32 * g + 32, :],
                                    v_all[:, p_tc, h,
                                          half * 32:(half + 1) * 32],
                                    p_es[sbq][:, hh * 512:(hh + 1) * 512],
                                    start=(p_tc == 0), stop=(p_tc == TC - 1),
                                    tile_position=(0, 32 * g),
                                    skip_group_check=True,
                                )
                        for hh in range(2):
                            qq = hh * 2 + sbq
                            nc.tensor.matmul(
                                den[32 * qq:32 * qq + 1, :],
                                ones[:, 0:1],
                                p_es[sbq][:, hh * 512:(hh + 1) * 512],
                                start=(p_tc == 0), stop=(p_tc == TC - 1),
                                tile_position=(0, 32 * qq),
                                skip_group_check=True,
                            )

                    prev = None
                    for tc in range(TC):
                        t_sl = slice(tc * 128, (tc + 1) * 128)
                        es = []
                        for sbq in range(2):
                            if prev is not None:
                                emit_av_den(prev[0], prev[1], sbq)
                            st = psS.tile([128, 1024], F32, tag="st",
                                          name=f"st{pair}_{sB}_{tc}_{sbq}")
                            s_sl = slice(sB * 1024 + sbq * 512,
                                         sB * 1024 + (sbq + 1) * 512)
                            for hh in range(2):
                                p0 = hh * 64
                                nc.tensor.matmul(
                                    st[:, hh * 512:(hh + 1) * 512],
                                    k_all[p0:p0 + 64, pair, t_sl],
                                    q_all[p0:p0 + 64, pair, s_sl],
                                    start=True, stop=True,
                                    tile_position=(p0, 0),
                                )
                            e = expp.tile([128, 1024], BF16, tag="e",
                                          name=f"e{pair}_{sB}_{tc}_{sbq}")
                            nc.scalar.activation(
                                e[:], st[:],
                                mybir.ActivationFunctionType.Exp,
                                scale=0.125,
                            )
                            es.append(e)
                        if interleave:
                            pop_proj(3 if pair == 0 else 2)
                        for _ in range(2):
                            if norm_steps:
                                norm_steps.pop(0)()
                        prev = (es, tc)
                    for sbq in range(2):
                        emit_av_den(prev[0], prev[1], sbq)

                    if DEBUG_RAW and pair == 0 and sB == 0:
                        dbg = attn.tile([128, 3, 512], F32, tag="dbg", bufs=1)
                        nc.vector.tensor_copy(dbg[:, 0, :], pvs[0][:])
                        nc.vector.tensor_copy(dbg[:, 1, :], pvs[1][:])
                        nc.vector.tensor_copy(dbg[:, 2, :], den[:])
                        nc.sync.dma_start(
                            out.ap()[0:384, 0:512].rearrange(
                                "(a p) b -> p a b", p=128),
                            dbg[:])
                    # ------- evacuate + defer normalize (pair, sB) -------
                    # Fast PSUM->SBUF copies free the pv/den banks for the
                    # next block; the recip/broadcast/mul chain is deferred
                    # into the next block's tc loop via norm_steps.
                    # (partition_broadcast to a non-zero output base partition
                    # mis-executes on HW: broadcast into base-0 [64,...] tiles
                    # only, and split the muls per head half.)
                    stage = attn.tile([128, 3, 512], F32, tag="stage",
                                      bufs=2, name=f"stage{pair}_{sB}")
                    nc.vector.tensor_copy(stage[:, 0, :], pvs[0][:])
                    nc.vector.tensor_copy(stage[:, 1, :], pvs[1][:])
                    nc.vector.tensor_copy(stage[:, 2, :], den[:])

                    def _norm_steps(stage, pair, sB):
                        recips = attn.tile([1, 4, 512], BF16, tag="recip",
                                           bufs=2, name=f"rc{pair}_{sB}")
                        rbc = attn.tile([64, 2, 512], BF16, tag="rbc",
                                        bufs=2, name=f"rb{pair}_{sB}")
                        rbf = attn.tile([128, 2, 512], BF16, tag="rbf",
                                        bufs=2, name=f"rf{pair}_{sB}")
                        def _recip(qq, recips=recips):
                            with nc.allow_low_precision(
                                    reason="softmax denom recip in bf16"):
                                nc.vector.reciprocal(
                                    recips[:, qq, :],
                                    stage[32 * qq:32 * qq + 1, 2, :])
                        for qq in range(4):
                            hh, sbq = qq // 2, qq % 2
                            yield lambda qq=qq: _recip(qq)
                            if hh == 0:
                                yield lambda sbq=sbq, qq=qq: (
                                    nc.gpsimd.partition_broadcast(
                                        rbf[0:64, sbq, :], recips[:, qq, :]))
                            else:
                                yield lambda sbq=sbq, qq=qq: (
                                    nc.gpsimd.partition_broadcast(
                                        rbc[:, sbq, :], recips[:, qq, :]))
                                yield lambda sbq=sbq: (
                                    nc.gpsimd.tensor_copy(
                                        rbf[64:128, sbq, :], rbc[:, sbq, :]))
                        for sbq in range(2):
                            s_sl = slice(sB * 1024 + sbq * 512,
                                         sB * 1024 + (sbq + 1) * 512)
                            for hh in range(2):
                                yield lambda sbq=sbq, hh=hh, s_sl=s_sl: (
                                    nc.vector.tensor_mul(
                                        o_all[hh * 64:(hh + 1) * 64, pair, s_sl],
                                        stage[hh * 64:(hh + 1) * 64, sbq, :],
                                        rbf[hh * 64:(hh + 1) * 64, sbq, :]))

                    norm_steps.extend(_norm_steps(stage, pair, sB))

        for step in norm_steps:
            step()

        if "c" not in phases:
            with tc_.tile_pool(name="dump2", bufs=1) as dump2:
                dt2 = dump2.tile([128, 512], F32, tag="dump2")
                nc.vector.tensor_copy(dt2[:], o_all[:, 0, 0:512])
                nc.sync.dma_start(out.ap()[0:128, 0:512], dt2[:])
            return

        # --------------------- output projection (partial) -----------------
        with (
            tc_.tile_pool(name="outp", bufs=3) as outp,
            tc_.tile_pool(name="psC", bufs=2, space="PSUM") as psC,
        ):
            for sc in range(S // 128):
                for db in range(2):
                    ps = psC.tile([128, 512], F32, tag="psC")
                    for dc in range(DC):
                        nc.tensor.matmul(
                            ps[:],
                            o_all[:, dc, sc * 128:(sc + 1) * 128],
                            w0_t[:, dc, db * 512:(db + 1) * 512],
                            start=(dc == 0), stop=(dc == DC - 1),
                        )
                    ot = outp.tile([128, 512], F32, tag="ot")
                    nc.vector.tensor_copy(ot[:], ps[:])
                    nc.sync.dma_start(
                        out.ap()[sc * 128:(sc + 1) * 128,
                                 db * 512:(db + 1) * 512],
                        ot[:],
                    )


_NC_CACHE = {}


def _get_nc(loop_n=1):
    if loop_n not in _NC_CACHE:
        _NC_CACHE[loop_n] = build(loop_n)
    return _NC_CACHE[loop_n]


def _prep_in_maps(q, k, v, Wq, bq, Wk, bk, Wv, bv, W0, b0):
    def bt(x):  # bf16, C-contiguous transpose
        return np.ascontiguousarray(np.asarray(x, np.float32).T.astype(NP_BF16))

    xs = [(bt(q[b]), bt(k[b]), bt(v[b])) for b in range(4)]
    Wq32 = np.asarray(Wq, np.float32)
    Wk32 = np.asarray(Wk, np.float32)
    Wv32 = np.asarray(Wv, np.float32)
    W032 = np.asarray(W0, np.float32)
    bq32 = np.asarray(bq, np.float32)

    in_maps = []
    for c in range(8):
        b, hg = c // 2, c % 2
        sl = slice(hg * DS, (hg + 1) * DS)
        xq_c, xk_c, xv_c = xs[b]
        in_maps.append({
            "xq": xq_c, "xk": xk_c, "xv": xv_c,
            "wq": np.ascontiguousarray(Wq32[sl].T).astype(NP_BF16),
            "wk": np.ascontiguousarray(Wk32[sl].T).astype(NP_BF16),
            "wv": np.ascontiguousarray(Wv32[sl].T).astype(NP_BF16),
            "w0": np.ascontiguousarray(W032[:, sl].T).astype(NP_BF16),
            "bq": np.ascontiguousarray(bq32[sl]),
        })
    return in_maps


def kernel(q, k, v, mask, Wq, bq, Wk, bk, Wv, bv, W0, b0):
    nc = _get_nc(1)
    in_maps = _prep_in_maps(q, k, v, Wq, bq, Wk, bk, Wv, bv, W0, b0)
    res = run_bass_kernel_spmd(nc, in_maps, core_ids=list(range(8)))
    B = q.shape[0]
    # host reduce: sum the two head-group partials + b0 + W0 @ bv
    b0e = (
        np.asarray(b0, np.float64)
        + np.asarray(W0, np.float64) @ np.asarray(bv, np.float64)
    ).astype(np.float32)
    outv = np.empty((B, S, D), np.float32)
    for b in range(B):
        outv[b] = res.results[2 * b]["out"] + res.results[2 * b + 1]["out"] + b0e
    return outv
